# revision 21
# baseline (speedup 1.0000x reference)
"""Trainium2 Bass kernel for nn_AggregationLoss (segment_reduce).

Data-parallel over batch: 32 samples -> 8 cores x 4 samples.

Per-sample algorithm (P = 65536 pixels as [128 part x 512 free], MAX_T = 16):
  - one-hot planes OH_K/OH_T built with 4x-mode tensor_scalar is_equal (bf16)
  - segment sums k_sum/k_cnt via 512 accumulating matmuls:
      lhsT = [s0..s3|ones] strided view [128,5], rhs = OH_K_j [128,16]
  - G = k_sum/max(k_cnt,1); per-pixel gather of (G0,G1) and (G2,G3) by
    packing two bf16 values into one fp32 and accumulating
    mask*packedval over the 16 disjoint masks (exact: adds of +0.0)
  - loss chain on ACT using only the ln/exp table set (sqrt = exp(0.5*ln))
  - inst_sum/t_cnt via a second 512-matmul pass (lhsT = [ones|loss])
  - final = sum_t valid_t * inst_sum_t / (max(t_cnt,1)*max(n_valid,1))
"""

import sys

sys.path.insert(0, "/opt/trn_rl_repo")

import numpy as np  # noqa: E402

import concourse.bacc as bacc  # noqa: E402
import concourse.bass as bass  # noqa: E402
import concourse.mybir as mybir  # noqa: E402
from concourse import tile  # noqa: E402
from concourse.bass_utils import run_bass_kernel_spmd  # noqa: E402
from concourse.hw_specs import get_activation_tables  # noqa: E402

F32 = mybir.dt.float32
BF16 = mybir.dt.bfloat16
I32 = mybir.dt.int32
A = mybir.AluOpType
AF = mybir.ActivationFunctionType

NCORES = 8
NSAMP = 4  # samples per core
NT = 16  # instance ids
NS = NT - 1  # non-background instance ids (t = 1..15)
PJ = 512  # free size of a [128, 512] pixel tile


def _plane(t, b):
    """block b of a [128, nb*512] tile"""
    return t[:, b * PJ : (b + 1) * PJ]


def phase1(nc, pools, preds, targets, n):
    big, med, small, (psa_pool, psc_pool), ohpool, persist = pools

    simf = big.tile([128, 4 * PJ], F32, tag="simf")
    for c in range(4):
        nc.sync.dma_start(
            _plane(simf, c), preds[n, 2 + c].rearrange("(p a) b -> p (a b)", p=128)
        )
    idsT_i = med.tile([128, PJ], I32, tag="idsTi")
    idsK_i = med.tile([128, PJ], I32, tag="idsKi")
    nc.sync.dma_start(idsT_i[:], targets[n, 0].rearrange("(p a) b -> p (a b)", p=128))
    nc.sync.dma_start(idsK_i[:], targets[n, 1].rearrange("(p a) b -> p (a b)", p=128))

    # sim6 blocks: 0..3 = sim bf16, 4 = ones, 5 = loss (later); converts on
    # the otherwise idle GPSIMD engine
    sim6 = persist.tile([128, 6 * PJ], BF16, tag="sim6")
    for c in range(4):
        nc.gpsimd.tensor_copy(_plane(sim6, c), _plane(simf, c))
    nc.gpsimd.memset(_plane(sim6, 4), 1.0)

    idsT = med.tile([128, PJ], BF16, tag="idsT")
    idsK = med.tile([128, PJ], BF16, tag="idsK")
    nc.gpsimd.tensor_copy(idsT[:], idsT_i[:])
    nc.gpsimd.tensor_copy(idsK[:], idsK_i[:])

    # planes for t = 1..15 only: t=0 (background) is excluded from the loss
    # and G[0] is never gathered for a pixel whose loss survives
    OHK = ohpool.tile([128, NS * PJ], BF16, tag="OHK")
    OHT = persist.tile([128, NS * PJ], BF16, tag="OHT")
    for i in range(NS):
        t = i + 1
        nc.vector.tensor_scalar(_plane(OHK, i), idsK[:], float(t), None, A.is_equal)
        nc.vector.tensor_scalar(_plane(OHT, i), idsT[:], float(t), None, A.is_equal)

    psA = psa_pool.tile([5, NS], F32, tag="psA")
    lhsA = sim6[:].rearrange("p (b j) -> p j b", b=6)  # [128, 512, 6]
    rhsK = OHK[:].rearrange("p (t j) -> p j t", t=NS)  # [128, 512, 15]
    for j in range(PJ):
        nc.tensor.matmul(
            psA[:],
            lhsA[:, j : j + 1, 0:5],
            rhsK[:, j : j + 1, :],
            start=(j == 0),
            stop=(j == PJ - 1),
        )
    return dict(sim6=sim6, OHT=OHT, psA=psA, lhsA=lhsA)


def phase2(nc, pools, st):
    big, med, small, (psa_pool, psc_pool), ohpool, persist = pools
    sim6, OHT, psA = st["sim6"], st["OHT"], st["psA"]

    stA = small.tile([5, NS], F32, tag="stA")
    nc.vector.tensor_copy(stA[:], psA[:])
    flatA = small.tile([1, 5 * NS], F32, tag="flatA")
    nc.sync.dma_start(flatA[:, 0 : 5 * NS], stA[:])
    k_cnt = flatA[:, 4 * NS : 5 * NS]

    kc1 = small.tile([1, NS], F32, tag="kc1")
    nc.vector.tensor_scalar(kc1[:], k_cnt, 1.0, None, A.max)
    rk = small.tile([1, NS], F32, tag="rk")
    nc.vector.reciprocal(rk[:], kc1[:])
    Gflat = small.tile([1, 4 * NS], F32, tag="Gflat")
    for c in range(4):
        nc.vector.tensor_tensor(
            Gflat[:, c * NS : (c + 1) * NS],
            flatA[:, c * NS : (c + 1) * NS],
            rk[:],
            A.mult,
        )
    Gbf = small.tile([1, 4 * NS], BF16, tag="Gbf")
    nc.vector.tensor_copy(Gbf[:], Gflat[:])  # round to bf16

    # pack (G0,G1) and (G2,G3) pairs into fp32 by writing bf16 halves:
    # fp32 little-endian: high 2 bytes = bf16 element index 1 of the pair.
    V01 = small.tile([1, NS], F32, tag="V01")
    V23 = small.tile([1, NS], F32, tag="V23")
    for V, chi, clo in ((V01, 0, 1), (V23, 2, 3)):
        vb = V[:].bitcast(BF16).rearrange("p (j two) -> p j two", two=2)
        nc.vector.tensor_copy(vb[:, :, 1:2], Gbf[:, chi * NS : (chi + 1) * NS])
        nc.vector.tensor_copy(vb[:, :, 0:1], Gbf[:, clo * NS : (clo + 1) * NS])
    V01b = med.tile([128, NS], F32, tag="V01b")
    V23b = med.tile([128, NS], F32, tag="V23b")
    nc.gpsimd.partition_broadcast(V01b[:], V01[:])
    nc.gpsimd.partition_broadcast(V23b[:], V23[:])

    W01 = big.tile([128, PJ], F32, tag="W01")
    W23 = big.tile([128, PJ], F32, tag="W23")
    nc.vector.tensor_scalar(W01[:], _plane(OHT, 0), V01b[:, 0:1], None, A.mult)
    nc.vector.tensor_scalar(W23[:], _plane(OHT, 0), V23b[:, 0:1], None, A.mult)
    for i in range(1, NS):
        nc.vector.scalar_tensor_tensor(
            W01[:], _plane(OHT, i), V01b[:, i : i + 1], W01[:], A.mult, A.add
        )
        nc.vector.scalar_tensor_tensor(
            W23[:], _plane(OHT, i), V23b[:, i : i + 1], W23[:], A.mult, A.add
        )

    # per-pixel gathered means as bf16 views of the packed fp32 accumulators
    w01v = W01[:].bitcast(BF16).rearrange("p (j two) -> p j two", two=2)
    w23v = W23[:].bitcast(BF16).rearrange("p (j two) -> p j two", two=2)
    wviews = (w01v[:, :, 1:2], w01v[:, :, 0:1], w23v[:, :, 1:2], w23v[:, :, 0:1])

    a4 = big.tile([128, 4 * PJ], BF16, tag="a4")
    for c in range(4):
        eng = nc.gpsimd if c < 2 else nc.vector
        eng.tensor_tensor(_plane(a4, c), _plane(sim6, c), wviews[c], A.subtract)

    sq4 = big.tile([128, 4 * PJ], BF16, tag="sq4")
    nc.vector.tensor_tensor(sq4[:], a4[:], a4[:], A.mult)
    s2 = med.tile([128, 2 * PJ], BF16, tag="s2")
    nc.vector.tensor_tensor(s2[:], sq4[:, 0 : 2 * PJ], sq4[:, 2 * PJ : 4 * PJ], A.add)
    d2 = med.tile([128, PJ], F32, tag="d2")
    nc.vector.tensor_tensor(d2[:], s2[:, 0:PJ], s2[:, PJ : 2 * PJ], A.add)

    # loss = ln(relu(sqrt(d2) - 0.5)^2 + 1); sqrt via exp(0.5*ln) keeps one
    # activation table set resident for the whole kernel
    lnd = med.tile([128, PJ], F32, tag="lnd")
    nc.scalar.activation(lnd[:], d2[:], AF.Ln)
    dd = med.tile([128, PJ], F32, tag="dd")
    nc.scalar.activation(dd[:], lnd[:], AF.Exp, scale=0.5)
    m = med.tile([128, PJ], BF16, tag="m")
    nc.scalar.activation(m[:], dd[:], AF.Relu, bias=-0.5)
    m2 = med.tile([128, PJ], BF16, tag="m2")
    nc.scalar.activation(m2[:], m[:], AF.Square)
    nc.scalar.activation(_plane(sim6, 5), m2[:], AF.Ln, bias=1.0)
    st["k_cnt_flatA"] = flatA


def phase3(nc, pools, st, out, n):
    big, med, small, (psa_pool, psc_pool), ohpool, persist = pools
    sim6, OHT, lhsA = st["sim6"], st["OHT"], st["lhsA"]
    flatA = st["k_cnt_flatA"]
    k_cnt = flatA[:, 4 * NS : 5 * NS]

    psC = psc_pool.tile([2, NS], F32, tag="psC")
    rhsT = OHT[:].rearrange("p (t j) -> p j t", t=NS)
    for j in range(PJ):
        nc.tensor.matmul(
            psC[:],
            lhsA[:, j : j + 1, 4:6],
            rhsT[:, j : j + 1, :],
            start=(j == 0),
            stop=(j == PJ - 1),
        )

    stC = small.tile([2, NS], F32, tag="stC")
    nc.vector.tensor_copy(stC[:], psC[:])
    flatC = small.tile([1, 2 * NS], F32, tag="flatC")
    nc.sync.dma_start(flatC[:, 0 : 2 * NS], stC[:])
    t_cnt = flatC[:, 0:NS]
    inst_sum = flatC[:, NS : 2 * NS]

    ka = small.tile([1, NS], F32, tag="ka")
    nc.vector.tensor_scalar(ka[:], k_cnt, 0.5, None, A.is_gt)
    ta = small.tile([1, NS], F32, tag="ta")
    nc.vector.tensor_scalar(ta[:], t_cnt, 0.5, None, A.is_gt)
    valid = small.tile([1, NS], F32, tag="valid")
    nc.vector.tensor_tensor(valid[:], ka[:], ta[:], A.mult)

    nv = small.tile([1, 1], F32, tag="nv")
    nc.vector.tensor_reduce(nv[:], valid[:], mybir.AxisListType.X, A.add)
    nv1 = small.tile([1, 1], F32, tag="nv1")
    nc.vector.tensor_scalar(nv1[:], nv[:], 1.0, None, A.max)
    rn = small.tile([1, 1], F32, tag="rn")
    nc.vector.reciprocal(rn[:], nv1[:])

    tc1 = small.tile([1, NS], F32, tag="tc1")
    nc.vector.tensor_scalar(tc1[:], t_cnt, 1.0, None, A.max)
    rt = small.tile([1, NS], F32, tag="rt")
    nc.vector.reciprocal(rt[:], tc1[:])

    wv = small.tile([1, NS], F32, tag="wv")
    nc.vector.tensor_tensor(wv[:], valid[:], rt[:], A.mult)
    wv2 = small.tile([1, NS], F32, tag="wv2")
    nc.vector.tensor_scalar(wv2[:], wv[:], rn[:, 0:1], None, A.mult)
    contrib = small.tile([1, NS], F32, tag="contrib")
    nc.vector.tensor_tensor(contrib[:], wv2[:], inst_sum, A.mult)
    fin = small.tile([1, 1], F32, tag="fin")
    nc.vector.tensor_reduce(fin[:], contrib[:], mybir.AxisListType.X, A.add)

    nc.sync.dma_start(out[n : n + 1], fin[:])


def build_nc():
    nc = bacc.Bacc("TRN2", target_bir_lowering=False, debug=False, num_devices=NCORES)
    # extra const APs used as activation biases
    for val in (-0.5,):
        t = nc.alloc_sbuf_tensor(f"const-f32-{val}", [128, 1], F32)
        nc.gpsimd.memset(t.ap(), val)
        nc.const_aps.aps[(F32, val)] = t.ap()
    preds = nc.declare_dram_parameter("preds", [NSAMP, 6, 256, 256], F32, isOutput=False)
    targets = nc.declare_dram_parameter(
        "targets", [NSAMP, 2, 256, 256], I32, isOutput=False
    )
    out = nc.declare_dram_parameter("out", [NSAMP], F32, isOutput=True)

    with tile.TileContext(nc) as tc:
        # pre-load the one activation table set containing every function we
        # use (ln/exp/relu/square); otherwise the auto-placement alternates
        # natural_log <-> exp_and_others, paying ~2.7us per switch
        tables = list(get_activation_tables(nc.m.arch))
        set_id = tables.index("natural_log_exp_and_others")
        nc.scalar.add_instruction(
            mybir.InstLoadActFuncSet(
                name=nc.get_next_instruction_name(),
                act_func_set_id=set_id,
                ins=[],
                outs=[],
            )
        )
        with (
            tc.tile_pool(name="big", bufs=2) as big,
            tc.tile_pool(name="med", bufs=2) as med,
            tc.tile_pool(name="small", bufs=4) as small,
            tc.tile_pool(name="psa", bufs=4, space="PSUM") as psa_pool,
            tc.tile_pool(name="psc", bufs=2, space="PSUM") as psc_pool,
            tc.tile_pool(name="ohk", bufs=2) as ohpool,
            tc.tile_pool(name="persist", bufs=4) as persist,
        ):
            pools = (big, med, small, (psa_pool, psc_pool), ohpool, persist)
            states = []
            for n in range(NSAMP):
                states.append(phase1(nc, pools, preds, targets, n))
            for n in range(NSAMP):
                phase2(nc, pools, states[n])
            for n in range(NSAMP):
                phase3(nc, pools, states[n], out, n)
    nc.finalize()
    return nc


_NC_CACHE = {}


def _get_nc():
    if "nc" not in _NC_CACHE:
        _NC_CACHE["nc"] = build_nc()
    return _NC_CACHE["nc"]


def kernel(preds: np.ndarray, targets: np.ndarray) -> np.ndarray:
    nc = _get_nc()
    in_maps = []
    for i in range(NCORES):
        in_maps.append(
            {
                "preds": np.ascontiguousarray(
                    preds[i * NSAMP : (i + 1) * NSAMP]
                ).astype(np.float32),
                "targets": np.ascontiguousarray(
                    targets[i * NSAMP : (i + 1) * NSAMP]
                ).astype(np.int32),
            }
        )
    res = run_bass_kernel_spmd(nc, in_maps, core_ids=list(range(NCORES)))
    outs = [res.results[i]["out"] for i in range(NCORES)]
    return np.concatenate(outs).astype(np.float32)


# revision 23
# speedup vs baseline: 1.0053x; 1.0053x over previous
"""Trainium2 Bass kernel for nn_AggregationLoss (segment_reduce).

Data-parallel over batch: 32 samples -> 8 cores x 4 samples.

Per-sample algorithm (P = 65536 pixels as [128 part x 512 free], MAX_T = 16):
  - one-hot planes OH_K/OH_T built with 4x-mode tensor_scalar is_equal (bf16)
  - segment sums k_sum/k_cnt via 512 accumulating matmuls:
      lhsT = [s0..s3|ones] strided view [128,5], rhs = OH_K_j [128,16]
  - G = k_sum/max(k_cnt,1); per-pixel gather of (G0,G1) and (G2,G3) by
    packing two bf16 values into one fp32 and accumulating
    mask*packedval over the 16 disjoint masks (exact: adds of +0.0)
  - loss chain on ACT using only the ln/exp table set (sqrt = exp(0.5*ln))
  - inst_sum/t_cnt via a second 512-matmul pass (lhsT = [ones|loss])
  - final = sum_t valid_t * inst_sum_t / (max(t_cnt,1)*max(n_valid,1))
"""

import sys

sys.path.insert(0, "/opt/trn_rl_repo")

import numpy as np  # noqa: E402

import concourse.bacc as bacc  # noqa: E402
import concourse.bass as bass  # noqa: E402
import concourse.mybir as mybir  # noqa: E402
from concourse import tile  # noqa: E402
from concourse.bass_utils import run_bass_kernel_spmd  # noqa: E402
from concourse.hw_specs import get_activation_tables  # noqa: E402

F32 = mybir.dt.float32
BF16 = mybir.dt.bfloat16
I32 = mybir.dt.int32
A = mybir.AluOpType
AF = mybir.ActivationFunctionType

NCORES = 8
NSAMP = 4  # samples per core
NT = 16  # instance ids
NS = NT - 1  # non-background instance ids (t = 1..15)
PJ = 512  # free size of a [128, 512] pixel tile


def _plane(t, b):
    """block b of a [128, nb*512] tile"""
    return t[:, b * PJ : (b + 1) * PJ]


def phase1(nc, pools, preds, targets, n):
    big, med, small, (psa_pool, psc_pool), ohpool, persist = pools

    simf = big.tile([128, 4 * PJ], F32, tag="simf")
    for c in range(4):
        nc.sync.dma_start(
            _plane(simf, c), preds[n, 2 + c].rearrange("(p a) b -> p (a b)", p=128)
        )
    idsT_i = med.tile([128, PJ], I32, tag="idsTi")
    idsK_i = med.tile([128, PJ], I32, tag="idsKi")
    nc.sync.dma_start(idsT_i[:], targets[n, 0].rearrange("(p a) b -> p (a b)", p=128))
    nc.sync.dma_start(idsK_i[:], targets[n, 1].rearrange("(p a) b -> p (a b)", p=128))

    # sim6 blocks: 0..3 = sim bf16, 4 = ones, 5 = loss (later); converts on
    # the otherwise idle GPSIMD engine
    sim6 = persist.tile([128, 6 * PJ], BF16, tag="sim6")
    for c in range(4):
        nc.gpsimd.tensor_copy(_plane(sim6, c), _plane(simf, c))
    nc.gpsimd.memset(_plane(sim6, 4), 1.0)

    idsT = med.tile([128, PJ], BF16, tag="idsT")
    idsK = med.tile([128, PJ], BF16, tag="idsK")
    nc.gpsimd.tensor_copy(idsT[:], idsT_i[:])
    nc.gpsimd.tensor_copy(idsK[:], idsK_i[:])

    # planes for t = 1..15 only: t=0 (background) is excluded from the loss
    # and G[0] is never gathered for a pixel whose loss survives
    OHK = ohpool.tile([128, NS * PJ], BF16, tag="OHK")
    OHT = persist.tile([128, NS * PJ], BF16, tag="OHT")
    for i in range(NS):
        t = i + 1
        nc.vector.tensor_scalar(_plane(OHK, i), idsK[:], float(t), None, A.is_equal)
        nc.vector.tensor_scalar(_plane(OHT, i), idsT[:], float(t), None, A.is_equal)

    psA = psa_pool.tile([5, NS], F32, tag="psA")
    lhsA = sim6[:].rearrange("p (b j) -> p j b", b=6)  # [128, 512, 6]
    rhsK = OHK[:].rearrange("p (t j) -> p j t", t=NS)  # [128, 512, 15]
    for j in range(PJ):
        nc.tensor.matmul(
            psA[:],
            lhsA[:, j : j + 1, 0:5],
            rhsK[:, j : j + 1, :],
            start=(j == 0),
            stop=(j == PJ - 1),
        )
    return dict(sim6=sim6, OHT=OHT, psA=psA, lhsA=lhsA)


def phase2(nc, pools, st):
    big, med, small, (psa_pool, psc_pool), ohpool, persist = pools
    sim6, OHT, psA = st["sim6"], st["OHT"], st["psA"]

    stA = small.tile([5, NS], F32, tag="stA")
    nc.vector.tensor_copy(stA[:], psA[:])
    flatA = small.tile([1, 5 * NS], F32, tag="flatA")
    nc.sync.dma_start(flatA[:, 0 : 5 * NS], stA[:])
    k_cnt = flatA[:, 4 * NS : 5 * NS]

    kc1 = small.tile([1, NS], F32, tag="kc1")
    nc.vector.tensor_scalar(kc1[:], k_cnt, 1.0, None, A.max)
    rk = small.tile([1, NS], F32, tag="rk")
    nc.vector.reciprocal(rk[:], kc1[:])
    Gflat = small.tile([1, 4 * NS], F32, tag="Gflat")
    for c in range(4):
        nc.gpsimd.tensor_tensor(
            Gflat[:, c * NS : (c + 1) * NS],
            flatA[:, c * NS : (c + 1) * NS],
            rk[:],
            A.mult,
        )
    Gbf = small.tile([1, 4 * NS], BF16, tag="Gbf")
    nc.vector.tensor_copy(Gbf[:], Gflat[:])  # round to bf16

    # pack (G0,G1) and (G2,G3) pairs into fp32 by writing bf16 halves:
    # fp32 little-endian: high 2 bytes = bf16 element index 1 of the pair.
    V01 = small.tile([1, NS], F32, tag="V01")
    V23 = small.tile([1, NS], F32, tag="V23")
    for V, chi, clo in ((V01, 0, 1), (V23, 2, 3)):
        vb = V[:].bitcast(BF16).rearrange("p (j two) -> p j two", two=2)
        nc.vector.tensor_copy(vb[:, :, 1:2], Gbf[:, chi * NS : (chi + 1) * NS])
        nc.vector.tensor_copy(vb[:, :, 0:1], Gbf[:, clo * NS : (clo + 1) * NS])
    V01b = med.tile([128, NS], F32, tag="V01b")
    V23b = med.tile([128, NS], F32, tag="V23b")
    nc.gpsimd.partition_broadcast(V01b[:], V01[:])
    nc.gpsimd.partition_broadcast(V23b[:], V23[:])

    W01 = big.tile([128, PJ], F32, tag="W01")
    W23 = big.tile([128, PJ], F32, tag="W23")
    nc.vector.tensor_scalar(W01[:], _plane(OHT, 0), V01b[:, 0:1], None, A.mult)
    nc.vector.tensor_scalar(W23[:], _plane(OHT, 0), V23b[:, 0:1], None, A.mult)
    for i in range(1, NS):
        nc.vector.scalar_tensor_tensor(
            W01[:], _plane(OHT, i), V01b[:, i : i + 1], W01[:], A.mult, A.add
        )
        nc.vector.scalar_tensor_tensor(
            W23[:], _plane(OHT, i), V23b[:, i : i + 1], W23[:], A.mult, A.add
        )

    # per-pixel gathered means as bf16 views of the packed fp32 accumulators
    w01v = W01[:].bitcast(BF16).rearrange("p (j two) -> p j two", two=2)
    w23v = W23[:].bitcast(BF16).rearrange("p (j two) -> p j two", two=2)
    wviews = (w01v[:, :, 1:2], w01v[:, :, 0:1], w23v[:, :, 1:2], w23v[:, :, 0:1])

    a4 = big.tile([128, 4 * PJ], BF16, tag="a4")
    for c in range(4):
        eng = nc.gpsimd if c < 2 else nc.vector
        eng.tensor_tensor(_plane(a4, c), _plane(sim6, c), wviews[c], A.subtract)

    sq4 = big.tile([128, 4 * PJ], BF16, tag="sq4")
    nc.vector.tensor_tensor(sq4[:], a4[:], a4[:], A.mult)
    s2 = med.tile([128, 2 * PJ], BF16, tag="s2")
    nc.vector.tensor_tensor(s2[:], sq4[:, 0 : 2 * PJ], sq4[:, 2 * PJ : 4 * PJ], A.add)
    d2 = med.tile([128, PJ], F32, tag="d2")
    nc.vector.tensor_tensor(d2[:], s2[:, 0:PJ], s2[:, PJ : 2 * PJ], A.add)

    # loss = ln(relu(sqrt(d2) - 0.5)^2 + 1); sqrt via exp(0.5*ln) keeps one
    # activation table set resident for the whole kernel
    lnd = med.tile([128, PJ], F32, tag="lnd")
    nc.scalar.activation(lnd[:], d2[:], AF.Ln)
    dd = med.tile([128, PJ], F32, tag="dd")
    nc.scalar.activation(dd[:], lnd[:], AF.Exp, scale=0.5)
    m = med.tile([128, PJ], BF16, tag="m")
    nc.scalar.activation(m[:], dd[:], AF.Relu, bias=-0.5)
    m2 = med.tile([128, PJ], BF16, tag="m2")
    nc.scalar.activation(m2[:], m[:], AF.Square)
    nc.scalar.activation(_plane(sim6, 5), m2[:], AF.Ln, bias=1.0)
    st["k_cnt_flatA"] = flatA


def phase3(nc, pools, st, out, n):
    big, med, small, (psa_pool, psc_pool), ohpool, persist = pools
    sim6, OHT, lhsA = st["sim6"], st["OHT"], st["lhsA"]
    flatA = st["k_cnt_flatA"]
    k_cnt = flatA[:, 4 * NS : 5 * NS]

    psC = psc_pool.tile([2, NS], F32, tag="psC")
    rhsT = OHT[:].rearrange("p (t j) -> p j t", t=NS)
    for j in range(PJ):
        nc.tensor.matmul(
            psC[:],
            lhsA[:, j : j + 1, 4:6],
            rhsT[:, j : j + 1, :],
            start=(j == 0),
            stop=(j == PJ - 1),
        )

    stC = small.tile([2, NS], F32, tag="stC")
    nc.vector.tensor_copy(stC[:], psC[:])
    flatC = small.tile([1, 2 * NS], F32, tag="flatC")
    nc.sync.dma_start(flatC[:, 0 : 2 * NS], stC[:])
    t_cnt = flatC[:, 0:NS]
    inst_sum = flatC[:, NS : 2 * NS]

    ka = small.tile([1, NS], F32, tag="ka")
    nc.vector.tensor_scalar(ka[:], k_cnt, 0.5, None, A.is_gt)
    ta = small.tile([1, NS], F32, tag="ta")
    nc.vector.tensor_scalar(ta[:], t_cnt, 0.5, None, A.is_gt)
    valid = small.tile([1, NS], F32, tag="valid")
    nc.vector.tensor_tensor(valid[:], ka[:], ta[:], A.mult)

    nv = small.tile([1, 1], F32, tag="nv")
    nc.vector.tensor_reduce(nv[:], valid[:], mybir.AxisListType.X, A.add)
    nv1 = small.tile([1, 1], F32, tag="nv1")
    nc.vector.tensor_scalar(nv1[:], nv[:], 1.0, None, A.max)
    rn = small.tile([1, 1], F32, tag="rn")
    nc.vector.reciprocal(rn[:], nv1[:])

    tc1 = small.tile([1, NS], F32, tag="tc1")
    nc.vector.tensor_scalar(tc1[:], t_cnt, 1.0, None, A.max)
    rt = small.tile([1, NS], F32, tag="rt")
    nc.vector.reciprocal(rt[:], tc1[:])

    wv = small.tile([1, NS], F32, tag="wv")
    nc.vector.tensor_tensor(wv[:], valid[:], rt[:], A.mult)
    wv2 = small.tile([1, NS], F32, tag="wv2")
    nc.vector.tensor_scalar(wv2[:], wv[:], rn[:, 0:1], None, A.mult)
    contrib = small.tile([1, NS], F32, tag="contrib")
    nc.vector.tensor_tensor(contrib[:], wv2[:], inst_sum, A.mult)
    fin = small.tile([1, 1], F32, tag="fin")
    nc.vector.tensor_reduce(fin[:], contrib[:], mybir.AxisListType.X, A.add)

    nc.sync.dma_start(out[n : n + 1], fin[:])


def build_nc():
    nc = bacc.Bacc("TRN2", target_bir_lowering=False, debug=False, num_devices=NCORES)
    # extra const APs used as activation biases
    for val in (-0.5,):
        t = nc.alloc_sbuf_tensor(f"const-f32-{val}", [128, 1], F32)
        nc.gpsimd.memset(t.ap(), val)
        nc.const_aps.aps[(F32, val)] = t.ap()
    preds = nc.declare_dram_parameter("preds", [NSAMP, 6, 256, 256], F32, isOutput=False)
    targets = nc.declare_dram_parameter(
        "targets", [NSAMP, 2, 256, 256], I32, isOutput=False
    )
    out = nc.declare_dram_parameter("out", [NSAMP], F32, isOutput=True)

    with tile.TileContext(nc) as tc:
        # pre-load the one activation table set containing every function we
        # use (ln/exp/relu/square); otherwise the auto-placement alternates
        # natural_log <-> exp_and_others, paying ~2.7us per switch
        tables = list(get_activation_tables(nc.m.arch))
        set_id = tables.index("natural_log_exp_and_others")
        nc.scalar.add_instruction(
            mybir.InstLoadActFuncSet(
                name=nc.get_next_instruction_name(),
                act_func_set_id=set_id,
                ins=[],
                outs=[],
            )
        )
        with (
            tc.tile_pool(name="big", bufs=2) as big,
            tc.tile_pool(name="med", bufs=2) as med,
            tc.tile_pool(name="small", bufs=4) as small,
            tc.tile_pool(name="psa", bufs=4, space="PSUM") as psa_pool,
            tc.tile_pool(name="psc", bufs=2, space="PSUM") as psc_pool,
            tc.tile_pool(name="ohk", bufs=2) as ohpool,
            tc.tile_pool(name="persist", bufs=4) as persist,
        ):
            pools = (big, med, small, (psa_pool, psc_pool), ohpool, persist)
            states = []
            for n in range(NSAMP):
                states.append(phase1(nc, pools, preds, targets, n))
            for n in range(NSAMP):
                phase2(nc, pools, states[n])
            for n in range(NSAMP):
                phase3(nc, pools, states[n], out, n)
    nc.finalize()
    return nc


_NC_CACHE = {}


def _get_nc():
    if "nc" not in _NC_CACHE:
        _NC_CACHE["nc"] = build_nc()
    return _NC_CACHE["nc"]


def kernel(preds: np.ndarray, targets: np.ndarray) -> np.ndarray:
    nc = _get_nc()
    in_maps = []
    for i in range(NCORES):
        in_maps.append(
            {
                "preds": np.ascontiguousarray(
                    preds[i * NSAMP : (i + 1) * NSAMP]
                ).astype(np.float32),
                "targets": np.ascontiguousarray(
                    targets[i * NSAMP : (i + 1) * NSAMP]
                ).astype(np.int32),
            }
        )
    res = run_bass_kernel_spmd(nc, in_maps, core_ids=list(range(NCORES)))
    outs = [res.results[i]["out"] for i in range(NCORES)]
    return np.concatenate(outs).astype(np.float32)


# revision 27
# speedup vs baseline: 1.0401x; 1.0346x over previous
"""Trainium2 Bass kernel for nn_AggregationLoss (segment_reduce).

Data-parallel over batch: 32 samples -> 8 cores x 4 samples.

Per-sample algorithm (P = 65536 pixels as [128 part x 512 free], MAX_T = 16):
  - one-hot planes OH_K/OH_T built with 4x-mode tensor_scalar is_equal (bf16)
  - segment sums k_sum/k_cnt via 512 accumulating matmuls:
      lhsT = [s0..s3|ones] strided view [128,5], rhs = OH_K_j [128,16]
  - G = k_sum/max(k_cnt,1); per-pixel gather of (G0,G1) and (G2,G3) by
    packing two bf16 values into one fp32 and accumulating
    mask*packedval over the 16 disjoint masks (exact: adds of +0.0)
  - loss chain on ACT using only the ln/exp table set (sqrt = exp(0.5*ln))
  - inst_sum/t_cnt via a second 512-matmul pass (lhsT = [ones|loss])
  - final = sum_t valid_t * inst_sum_t / (max(t_cnt,1)*max(n_valid,1))
"""

import sys

sys.path.insert(0, "/opt/trn_rl_repo")

import numpy as np  # noqa: E402

import concourse.bacc as bacc  # noqa: E402
import concourse.bass as bass  # noqa: E402
import concourse.mybir as mybir  # noqa: E402
from concourse import tile  # noqa: E402
from concourse.bass_utils import run_bass_kernel_spmd  # noqa: E402
from concourse.hw_specs import get_activation_tables  # noqa: E402

F32 = mybir.dt.float32
BF16 = mybir.dt.bfloat16
I32 = mybir.dt.int32
A = mybir.AluOpType
AF = mybir.ActivationFunctionType

NCORES = 8
NSAMP = 4  # samples per core
NT = 16  # instance ids
NS = NT - 1  # non-background instance ids (t = 1..15)
PJ = 512  # free size of a [128, 512] pixel tile


def _plane(t, b):
    """block b of a [128, nb*512] tile"""
    return t[:, b * PJ : (b + 1) * PJ]


def phase1(nc, pools, preds, targets, n):
    big, med, small, (psa_pool, psc_pool), ohpool, persist = pools

    # ids first: the one-hot planes (DVE) depend only on ids, so loading and
    # converting them before sim lets OH building overlap the sim converts
    idsT_i = med.tile([128, PJ], I32, tag="idsTi")
    idsK_i = med.tile([128, PJ], I32, tag="idsKi")
    nc.sync.dma_start(idsT_i[:], targets[n, 0].rearrange("(p a) b -> p (a b)", p=128))
    nc.sync.dma_start(idsK_i[:], targets[n, 1].rearrange("(p a) b -> p (a b)", p=128))
    idsT = med.tile([128, PJ], BF16, tag="idsT")
    idsK = med.tile([128, PJ], BF16, tag="idsK")
    nc.gpsimd.tensor_copy(idsK[:], idsK_i[:])
    nc.gpsimd.tensor_copy(idsT[:], idsT_i[:])

    simf = big.tile([128, 4 * PJ], F32, tag="simf")
    for c in range(4):
        nc.sync.dma_start(
            _plane(simf, c), preds[n, 2 + c].rearrange("(p a) b -> p (a b)", p=128)
        )
    # sim6 blocks: 0..3 = sim bf16, 4 = ones, 5 = loss (later); converts on
    # the otherwise idle GPSIMD engine
    sim6 = persist.tile([128, 6 * PJ], BF16, tag="sim6")
    for c in range(4):
        nc.gpsimd.tensor_copy(_plane(sim6, c), _plane(simf, c))
    nc.gpsimd.memset(_plane(sim6, 4), 1.0)

    # planes for t = 1..15 only: t=0 (background) is excluded from the loss
    # and G[0] is never gathered for a pixel whose loss survives
    OHK = ohpool.tile([128, NS * PJ], BF16, tag="OHK")
    OHT = persist.tile([128, NS * PJ], BF16, tag="OHT")
    for i in range(NS):
        t = i + 1
        nc.vector.tensor_scalar(_plane(OHK, i), idsK[:], float(t), None, A.is_equal)
        nc.vector.tensor_scalar(_plane(OHT, i), idsT[:], float(t), None, A.is_equal)

    psA = psa_pool.tile([5, NS], F32, tag="psA")
    lhsA = sim6[:].rearrange("p (b j) -> p j b", b=6)  # [128, 512, 6]
    rhsK = OHK[:].rearrange("p (t j) -> p j t", t=NS)  # [128, 512, 15]
    for j in range(PJ):
        nc.tensor.matmul(
            psA[:],
            lhsA[:, j : j + 1, 0:5],
            rhsK[:, j : j + 1, :],
            start=(j == 0),
            stop=(j == PJ - 1),
        )
    return dict(sim6=sim6, OHT=OHT, psA=psA, lhsA=lhsA)


def phase2(nc, pools, st):
    big, med, small, (psa_pool, psc_pool), ohpool, persist = pools
    sim6, OHT, psA = st["sim6"], st["OHT"], st["psA"]

    stA = small.tile([5, NS], F32, tag="stA")
    nc.vector.tensor_copy(stA[:], psA[:])
    flatA = small.tile([1, 5 * NS], F32, tag="flatA")
    nc.sync.dma_start(flatA[:, 0 : 5 * NS], stA[:])
    k_cnt = flatA[:, 4 * NS : 5 * NS]

    kc1 = small.tile([1, NS], F32, tag="kc1")
    nc.vector.tensor_scalar(kc1[:], k_cnt, 1.0, None, A.max)
    rk = small.tile([1, NS], F32, tag="rk")
    nc.vector.reciprocal(rk[:], kc1[:])
    Gflat = small.tile([1, 4 * NS], F32, tag="Gflat")
    for c in range(4):
        nc.gpsimd.tensor_tensor(
            Gflat[:, c * NS : (c + 1) * NS],
            flatA[:, c * NS : (c + 1) * NS],
            rk[:],
            A.mult,
        )
    Gbf = small.tile([1, 4 * NS], BF16, tag="Gbf")
    nc.vector.tensor_copy(Gbf[:], Gflat[:])  # round to bf16

    # pack (G0,G1) and (G2,G3) pairs into fp32 by writing bf16 halves:
    # fp32 little-endian: high 2 bytes = bf16 element index 1 of the pair.
    V01 = small.tile([1, NS], F32, tag="V01")
    V23 = small.tile([1, NS], F32, tag="V23")
    for V, chi, clo in ((V01, 0, 1), (V23, 2, 3)):
        vb = V[:].bitcast(BF16).rearrange("p (j two) -> p j two", two=2)
        nc.vector.tensor_copy(vb[:, :, 1:2], Gbf[:, chi * NS : (chi + 1) * NS])
        nc.vector.tensor_copy(vb[:, :, 0:1], Gbf[:, clo * NS : (clo + 1) * NS])
    V01b = med.tile([128, NS], F32, tag="V01b")
    V23b = med.tile([128, NS], F32, tag="V23b")
    nc.gpsimd.partition_broadcast(V01b[:], V01[:])
    nc.gpsimd.partition_broadcast(V23b[:], V23[:])

    W01 = big.tile([128, PJ], F32, tag="W01")
    W23 = big.tile([128, PJ], F32, tag="W23")
    nc.vector.tensor_scalar(W01[:], _plane(OHT, 0), V01b[:, 0:1], None, A.mult)
    nc.vector.tensor_scalar(W23[:], _plane(OHT, 0), V23b[:, 0:1], None, A.mult)
    for i in range(1, NS):
        nc.vector.scalar_tensor_tensor(
            W01[:], _plane(OHT, i), V01b[:, i : i + 1], W01[:], A.mult, A.add
        )
        nc.vector.scalar_tensor_tensor(
            W23[:], _plane(OHT, i), V23b[:, i : i + 1], W23[:], A.mult, A.add
        )

    # per-pixel gathered means as bf16 views of the packed fp32 accumulators
    w01v = W01[:].bitcast(BF16).rearrange("p (j two) -> p j two", two=2)
    w23v = W23[:].bitcast(BF16).rearrange("p (j two) -> p j two", two=2)
    wviews = (w01v[:, :, 1:2], w01v[:, :, 0:1], w23v[:, :, 1:2], w23v[:, :, 0:1])

    a4 = big.tile([128, 4 * PJ], BF16, tag="a4")
    for c in range(4):
        eng = nc.gpsimd if c < 2 else nc.vector
        eng.tensor_tensor(_plane(a4, c), _plane(sim6, c), wviews[c], A.subtract)

    sq4 = big.tile([128, 4 * PJ], BF16, tag="sq4")
    nc.vector.tensor_tensor(sq4[:], a4[:], a4[:], A.mult)
    s2 = med.tile([128, 2 * PJ], BF16, tag="s2")
    nc.vector.tensor_tensor(s2[:], sq4[:, 0 : 2 * PJ], sq4[:, 2 * PJ : 4 * PJ], A.add)
    d2 = med.tile([128, PJ], BF16, tag="d2")
    nc.vector.tensor_tensor(d2[:], s2[:, 0:PJ], s2[:, PJ : 2 * PJ], A.add)

    # loss = ln(relu(sqrt(d2) - 0.5)^2 + 1); sqrt via exp(0.5*ln) keeps one
    # activation table set resident for the whole kernel
    lnd = med.tile([128, PJ], F32, tag="lnd")
    nc.scalar.activation(lnd[:], d2[:], AF.Ln)
    dd = med.tile([128, PJ], F32, tag="dd")
    nc.scalar.activation(dd[:], lnd[:], AF.Exp, scale=0.5)
    m = med.tile([128, PJ], BF16, tag="m")
    nc.scalar.activation(m[:], dd[:], AF.Relu, bias=-0.5)
    m2 = med.tile([128, PJ], BF16, tag="m2")
    nc.scalar.activation(m2[:], m[:], AF.Square)
    nc.scalar.activation(_plane(sim6, 5), m2[:], AF.Ln, bias=1.0)
    st["k_cnt_flatA"] = flatA


def phase3(nc, pools, st, out, n):
    big, med, small, (psa_pool, psc_pool), ohpool, persist = pools
    sim6, OHT, lhsA = st["sim6"], st["OHT"], st["lhsA"]
    flatA = st["k_cnt_flatA"]
    k_cnt = flatA[:, 4 * NS : 5 * NS]

    psC = psc_pool.tile([2, NS], F32, tag="psC")
    rhsT = OHT[:].rearrange("p (t j) -> p j t", t=NS)
    for j in range(PJ):
        nc.tensor.matmul(
            psC[:],
            lhsA[:, j : j + 1, 4:6],
            rhsT[:, j : j + 1, :],
            start=(j == 0),
            stop=(j == PJ - 1),
        )

    stC = small.tile([2, NS], F32, tag="stC")
    nc.vector.tensor_copy(stC[:], psC[:])
    flatC = small.tile([1, 2 * NS], F32, tag="flatC")
    nc.sync.dma_start(flatC[:, 0 : 2 * NS], stC[:])
    t_cnt = flatC[:, 0:NS]
    inst_sum = flatC[:, NS : 2 * NS]

    ka = small.tile([1, NS], F32, tag="ka")
    nc.vector.tensor_scalar(ka[:], k_cnt, 0.5, None, A.is_gt)
    ta = small.tile([1, NS], F32, tag="ta")
    nc.vector.tensor_scalar(ta[:], t_cnt, 0.5, None, A.is_gt)
    valid = small.tile([1, NS], F32, tag="valid")
    nc.vector.tensor_tensor(valid[:], ka[:], ta[:], A.mult)

    nv = small.tile([1, 1], F32, tag="nv")
    nc.vector.tensor_reduce(nv[:], valid[:], mybir.AxisListType.X, A.add)
    nv1 = small.tile([1, 1], F32, tag="nv1")
    nc.vector.tensor_scalar(nv1[:], nv[:], 1.0, None, A.max)
    rn = small.tile([1, 1], F32, tag="rn")
    nc.vector.reciprocal(rn[:], nv1[:])

    tc1 = small.tile([1, NS], F32, tag="tc1")
    nc.vector.tensor_scalar(tc1[:], t_cnt, 1.0, None, A.max)
    rt = small.tile([1, NS], F32, tag="rt")
    nc.vector.reciprocal(rt[:], tc1[:])

    wv = small.tile([1, NS], F32, tag="wv")
    nc.vector.tensor_tensor(wv[:], valid[:], rt[:], A.mult)
    wv2 = small.tile([1, NS], F32, tag="wv2")
    nc.vector.tensor_scalar(wv2[:], wv[:], rn[:, 0:1], None, A.mult)
    contrib = small.tile([1, NS], F32, tag="contrib")
    nc.vector.tensor_tensor(contrib[:], wv2[:], inst_sum, A.mult)
    fin = small.tile([1, 1], F32, tag="fin")
    nc.vector.tensor_reduce(fin[:], contrib[:], mybir.AxisListType.X, A.add)

    nc.sync.dma_start(out[n : n + 1], fin[:])


def build_nc():
    nc = bacc.Bacc("TRN2", target_bir_lowering=False, debug=False, num_devices=NCORES)
    # extra const APs used as activation biases
    for val in (-0.5,):
        t = nc.alloc_sbuf_tensor(f"const-f32-{val}", [128, 1], F32)
        nc.gpsimd.memset(t.ap(), val)
        nc.const_aps.aps[(F32, val)] = t.ap()
    preds = nc.declare_dram_parameter("preds", [NSAMP, 6, 256, 256], F32, isOutput=False)
    targets = nc.declare_dram_parameter(
        "targets", [NSAMP, 2, 256, 256], I32, isOutput=False
    )
    out = nc.declare_dram_parameter("out", [NSAMP], F32, isOutput=True)

    with tile.TileContext(nc) as tc:
        # pre-load the one activation table set containing every function we
        # use (ln/exp/relu/square); otherwise the auto-placement alternates
        # natural_log <-> exp_and_others, paying ~2.7us per switch
        tables = list(get_activation_tables(nc.m.arch))
        set_id = tables.index("natural_log_exp_and_others")
        nc.scalar.add_instruction(
            mybir.InstLoadActFuncSet(
                name=nc.get_next_instruction_name(),
                act_func_set_id=set_id,
                ins=[],
                outs=[],
            )
        )
        with (
            tc.tile_pool(name="big", bufs=2) as big,
            tc.tile_pool(name="med", bufs=2) as med,
            tc.tile_pool(name="small", bufs=4) as small,
            tc.tile_pool(name="psa", bufs=4, space="PSUM") as psa_pool,
            tc.tile_pool(name="psc", bufs=2, space="PSUM") as psc_pool,
            tc.tile_pool(name="ohk", bufs=2) as ohpool,
            tc.tile_pool(name="persist", bufs=4) as persist,
        ):
            pools = (big, med, small, (psa_pool, psc_pool), ohpool, persist)
            states = []
            for n in range(NSAMP):
                states.append(phase1(nc, pools, preds, targets, n))
            for n in range(NSAMP):
                phase2(nc, pools, states[n])
            for n in range(NSAMP):
                phase3(nc, pools, states[n], out, n)
    nc.finalize()
    return nc


_NC_CACHE = {}


def _get_nc():
    if "nc" not in _NC_CACHE:
        _NC_CACHE["nc"] = build_nc()
    return _NC_CACHE["nc"]


def kernel(preds: np.ndarray, targets: np.ndarray) -> np.ndarray:
    nc = _get_nc()
    in_maps = []
    for i in range(NCORES):
        in_maps.append(
            {
                "preds": np.ascontiguousarray(
                    preds[i * NSAMP : (i + 1) * NSAMP]
                ).astype(np.float32),
                "targets": np.ascontiguousarray(
                    targets[i * NSAMP : (i + 1) * NSAMP]
                ).astype(np.int32),
            }
        )
    res = run_bass_kernel_spmd(nc, in_maps, core_ids=list(range(NCORES)))
    outs = [res.results[i]["out"] for i in range(NCORES)]
    return np.concatenate(outs).astype(np.float32)
